# revision 1
# baseline (speedup 1.0000x reference)
"""Trainium2 Bass kernel for nn_GCLSTMModel_48868137894020 (v2).

Same algebraic reduction as v1 (H0 = C0 = 0 kills the cheb convs, the
forget gate, and peep[0/1]); per layer (d = 140 then 280), X = input:
  I = sigmoid(X @ W[0] + cb[0] + b[0])
  T = tanh   (X @ W[2] + cb[2] + b[2])
  C = I * T
  O = sigmoid(X @ W[3] + cb[3] + b[3] + peep[2] * C)
  X' = relu(O * tanh(C)) == max(tanh(C), 0) * O
then out = relu(X'' @ fc_w + fc_b).

v2 layout strategy (single-core program replicated on 8 cores):
  - EVERYTHING transposed: features on partitions, nodes (35) on the
    free dim, for both layers AND the FC, so no PE transposes are needed
    anywhere. Feature chunks of 128 are packed side by side in 36-column
    blocks of one tile: L1 [128, 72] (140 = 128+12), L2 [128, 108]
    (280 = 128+128+24).
  - All matmuls in bf16 (fp32 PSUM accumulation). Weights are cast to
    bf16 on the host; adj is 0/1 so its bf16 form is exact. Measured
    end-to-end rel err 2.4e-3 (gate is 2e-2).
  - Gate biases (cb+b) are folded into the matmul by extending the
    contraction with a constant-1 row (host-packed into the weights), so
    sigmoid/tanh run as ONE big activation per gate over all feature
    chunks at once instead of per chunk.
  - The O gate is split per chunk so each sigmoid fires as soon as its
    chunk's peephole STT lands, pipelining ACT against DVE.
  - x1T / x2T bf16 tiles double as the next matmul's rhs/lhsT, with a
    pre-memset ones row appended for the next bias fold.
  - The output DMA carries no waited semaphore: its ~2us flight is
    hidden under the compiler's fixed ~6.5us semaphore-reset epilogue.

Sharding: problem is tiny (N=35); all 8 cores run the identical program
on replicated inputs (no collectives), output taken from core 0.
"""

import sys

for _p in ("/opt/trn_rl_repo", "/opt/pypackages"):
    if _p not in sys.path:
        sys.path.append(_p)

from contextlib import ExitStack

import numpy as np
import ml_dtypes

import concourse.bacc as bacc
import concourse.bass as bass
import concourse.mybir as mybir
import concourse.tile as tile

F32 = mybir.dt.float32
BF16 = mybir.dt.bfloat16
AF = mybir.ActivationFunctionType
OP = mybir.AluOpType
GATES = (0, 2, 3)  # I, T (cell), O - forget gate (1) is dead
N = 35
D1 = 140
D2 = 280
N_CORES = 8
BF = ml_dtypes.bfloat16

# L1 feature chunks (140 = 128 + 12), L2 chunks (280 = 128 + 128 + 24).
C1 = ((0, 128), (128, 140))
C2 = ((0, 128), (128, 256), (256, 280))


def build_nc() -> bass.Bass:
    nc = bacc.Bacc()

    # w1pa: [36, 316] bf16; cols 0:36 = [adj; ones] (k=36 rhs for L1),
    # cols 36:176 = [W1[I]; bias], cols 176:316 = [W1[T]; bias].
    w1pa = nc.dram_tensor("w1pa", [36, 316], BF16, kind="ExternalInput")
    # w1pb: [128, 150] bf16: rows 0:36 cols 0:140 = [W1[O]; bias];
    # cols 140:150 carry the peep1[2]/peep2[2] per-partition scalars as
    # a bit-cast fp32 [128, 5] block (no separate peephole transfers).
    w1pb = nc.dram_tensor("w1pb", [128, 150], BF16, kind="ExternalInput")
    # W2 k-chunk0 (input features 0:128): gates I, T
    w2a = nc.dram_tensor("w2a", [128, 2, D2], BF16, kind="ExternalInput")
    # W2 k-chunk0: gate O
    w2b = nc.dram_tensor("w2b", [128, 1, D2], BF16, kind="ExternalInput")
    # W2 k-chunk1 (features 128:140 + bias row): [13, 3, 280]
    w2k1 = nc.dram_tensor("w2k1", [13, 3, D2], BF16, kind="ExternalInput")
    # FC: [fc_w; fc_b] row-chunks as [128, 3, 36]
    fcw = nc.dram_tensor("fcw", [128, 3, 36], BF16, kind="ExternalInput")
    out = nc.dram_tensor("out", [N, N], F32, kind="ExternalOutput")

    # persistent sbuf tensor: written by the relu inside the tile
    # context, DMA'd out AFTER the context (ordered by the context-exit
    # all-engine barrier) with a semaphore nothing waits on.
    out_sb = nc.alloc_sbuf_tensor("out_sbuf", [N, N], F32)
    out_sem = nc.alloc_semaphore("out_dma_sem")

    with ExitStack() as ctx:
        tc = ctx.enter_context(tile.TileContext(nc))
        sb = ctx.enter_context(tc.tile_pool(name="sb", bufs=1))
        ps1 = ctx.enter_context(tc.tile_pool(name="ps1", bufs=3, space="PSUM"))
        ps2 = ctx.enter_context(tc.tile_pool(name="ps2", bufs=4, space="PSUM"))

        # ---- input DMAs (issue order = queue order per engine) ----
        # All weight tensors ride the sync HWDGE ring in need-order; the
        # scalar/ACT queue is kept free so BOTH activation-table loads
        # run back-to-back at its head (each load is ~1.3us and blocks
        # later activations); small tensors ride the gpsimd SWDGE queue.
        w1pa_sb = sb.tile([36, 316], BF16, tag="w1pa")
        nc.sync.dma_start(out=w1pa_sb, in_=w1pa[:, :])
        w1pb_sb = sb.tile([128, 150], BF16, tag="w1pb")
        nc.sync.dma_start(out=w1pb_sb, in_=w1pb[:, :])
        w2a_sb = sb.tile([128, 2, D2], BF16, tag="w2a")
        nc.sync.dma_start(out=w2a_sb, in_=w2a[:, :, :])
        w2k1_sb = sb.tile([13, 3, D2], BF16, tag="w2k1")
        nc.sync.dma_start(out=w2k1_sb, in_=w2k1[:, :, :])
        w2b_sb = sb.tile([128, 1, D2], BF16, tag="w2b")
        nc.sync.dma_start(out=w2b_sb, in_=w2b[:, :, :])
        fcw_sb = sb.tile([128, 3, 36], BF16, tag="fcw")
        nc.sync.dma_start(out=fcw_sb, in_=fcw[:, :, :])

        adjp = w1pa_sb[:, 0:36]
        aux_v = w1pb_sb[:, 140:150].bitcast(F32)  # [128, 5] peep scalars

        # warm-up activations with no DMA deps: force BOTH act-table
        # loads to run during the DMA wait.
        warm_src = sb.tile([1, 2], F32, tag="warm_src")
        nc.vector.memset(warm_src[:, :], 0.25)
        warm = sb.tile([1, 2], F32, tag="warm")
        nc.scalar.activation(warm[0:1, 0:1], warm_src[0:1, 0:1], AF.Sigmoid)
        nc.scalar.activation(warm[0:1, 1:2], warm_src[0:1, 1:2], AF.Tanh)

        # ones rows for the bias folds: memset the whole chunk region
        # (partition offset must be 0-aligned); the gate STT later
        # overwrites all but the last row.
        x1T = sb.tile([128, 72], BF16, tag="x1T")
        nc.vector.memset(x1T[0:13, 36:72], 1.0)
        x2T = sb.tile([128, 108], BF16, tag="x2T")
        nc.vector.memset(x2T[0:25, 72:108], 1.0)


        # ---- layer 1 (transposed): psum banks [128, 72] ----
        p1 = {
            g: ps1.tile([128, 72], F32, tag="ps1", name=f"p1_{g}")
            for g in range(3)
        }
        l1_lhs = (
            lambda a, b: w1pa_sb[:, 36 + a : 36 + b],
            lambda a, b: w1pa_sb[:, 176 + a : 176 + b],
            lambda a, b: w1pb_sb[0:36, a:b],
        )
        for g in range(3):
            for ci, (a, b) in enumerate(C1):
                nc.tensor.matmul(
                    p1[g][0 : b - a, ci * 36 : ci * 36 + 36],
                    lhsT=l1_lhs[g](a, b),
                    rhs=adjp,
                    start=True,
                    stop=True,
                )
        gi1 = sb.tile([128, 72], BF16, tag="gi1")
        nc.scalar.activation(gi1, p1[0][:, :], AF.Sigmoid)
        gt1 = sb.tile([128, 72], BF16, tag="gt1")
        nc.scalar.activation(gt1, p1[1][:, :], AF.Tanh)
        c1 = sb.tile([128, 72], BF16, tag="c1")
        nc.vector.tensor_mul(c1, gi1, gt1)
        tc1 = sb.tile([128, 72], BF16, tag="tc1")
        nc.scalar.activation(tc1, c1, AF.Tanh)
        po1 = sb.tile([128, 72], BF16, tag="po1")
        for ci, (a, b) in enumerate(C1):
            cs = b - a
            nc.vector.scalar_tensor_tensor(
                po1[0:cs, ci * 36 : ci * 36 + 36],
                in0=c1[0:cs, ci * 36 : ci * 36 + 36],
                scalar=aux_v[0:cs, ci : ci + 1],
                in1=p1[2][0:cs, ci * 36 : ci * 36 + 36],
                op0=OP.mult, op1=OP.add,
            )
        # O sigmoid split per chunk: chunk0's fires as soon as its STT
        # lands, so x1T chunk0 (the big L2 matmul rhs) is ready earlier
        go1 = sb.tile([128, 72], BF16, tag="go1")
        nc.scalar.activation(go1[0:128, 0:36], po1[0:128, 0:36], AF.Sigmoid)
        nc.scalar.activation(go1[0:12, 36:72], po1[0:12, 36:72], AF.Sigmoid)
        # x1T = max(tanh(C), 0) * O
        nc.vector.scalar_tensor_tensor(
            x1T[0:128, 0:36], in0=tc1[0:128, 0:36], scalar=0.0,
            in1=go1[0:128, 0:36], op0=OP.max, op1=OP.mult,
        )
        nc.vector.scalar_tensor_tensor(
            x1T[0:12, 36:72], in0=tc1[0:12, 36:72], scalar=0.0,
            in1=go1[0:12, 36:72], op0=OP.max, op1=OP.mult,
        )

        # ---- layer 2 (transposed): psum banks [128, 108] ----
        p2 = {
            g: ps2.tile([128, 108], F32, tag="ps2", name=f"p2_{g}")
            for g in range(3)
        }
        w2k0_views = (w2a_sb[:, 0, :], w2a_sb[:, 1, :], w2b_sb[:, 0, :])
        # ONE open accumulation group per PSUM bank: close each region
        # (k0 then k1) before opening the next region in that bank.
        # Gate order I, T, O: O last so w2b (last weight DMA) can't
        # stall the PE queue ahead of the I/T work.
        for g in (0, 1, 2):
            for ci, (a, b) in enumerate(C2):
                nc.tensor.matmul(
                    p2[g][0 : b - a, ci * 36 : ci * 36 + 36],
                    lhsT=w2k0_views[g][:, a:b],
                    rhs=x1T[0:128, 0:36],
                    start=True,
                    stop=False,
                )
                nc.tensor.matmul(
                    p2[g][0 : b - a, ci * 36 : ci * 36 + 36],
                    lhsT=w2k1_sb[0:13, g, a:b],
                    rhs=x1T[0:13, 36:72],
                    start=False,
                    stop=True,
                )
        # every gate stage split into an A piece (chunks 0+1, ready one
        # third of the MM stream earlier) and a small c2 piece, so the
        # gate chain overlaps the tail of the matmul stream
        A = (slice(0, 128), slice(0, 72))
        B = (slice(0, 24), slice(72, 108))
        gi2 = sb.tile([128, 108], BF16, tag="gi2")
        nc.scalar.activation(gi2[A], p2[0][A], AF.Sigmoid)
        nc.scalar.activation(gi2[B], p2[0][B], AF.Sigmoid)
        gt2 = sb.tile([128, 108], BF16, tag="gt2")
        nc.scalar.activation(gt2[A], p2[1][A], AF.Tanh)
        nc.scalar.activation(gt2[B], p2[1][B], AF.Tanh)
        c2 = sb.tile([128, 108], BF16, tag="c2")
        nc.vector.tensor_mul(c2[A], gi2[A], gt2[A])
        nc.vector.tensor_mul(c2[B], gi2[B], gt2[B])
        tc2 = sb.tile([128, 108], BF16, tag="tc2")
        nc.scalar.activation(tc2[A], c2[A], AF.Tanh)
        nc.scalar.activation(tc2[B], c2[B], AF.Tanh)
        po2 = sb.tile([128, 108], BF16, tag="po2")
        for ci, (a, b) in enumerate(C2):
            cs = b - a
            nc.vector.scalar_tensor_tensor(
                po2[0:cs, ci * 36 : ci * 36 + 36],
                in0=c2[0:cs, ci * 36 : ci * 36 + 36],
                scalar=aux_v[0:cs, 2 + ci : 3 + ci],
                in1=p2[2][0:cs, ci * 36 : ci * 36 + 36],
                op0=OP.mult, op1=OP.add,
            )
        go2 = sb.tile([128, 108], BF16, tag="go2")
        nc.scalar.activation(go2[A], po2[A], AF.Sigmoid)
        nc.scalar.activation(go2[B], po2[B], AF.Sigmoid)
        nc.vector.scalar_tensor_tensor(
            x2T[A], in0=tc2[A], scalar=0.0, in1=go2[A],
            op0=OP.max, op1=OP.mult,
        )
        nc.vector.scalar_tensor_tensor(
            x2T[B], in0=tc2[B], scalar=0.0, in1=go2[B],
            op0=OP.max, op1=OP.mult,
        )
        psfc = ps2.tile([N, 36], F32, tag="ps2", name="psfc")
        nc.tensor.matmul(
            psfc, lhsT=x2T[0:128, 0:35], rhs=fcw_sb[0:128, 0, :],
            start=True, stop=False,
        )
        nc.tensor.matmul(
            psfc, lhsT=x2T[0:128, 36:71], rhs=fcw_sb[0:128, 1, :],
            start=False, stop=False,
        )
        nc.tensor.matmul(
            psfc, lhsT=x2T[0:25, 72:107], rhs=fcw_sb[0:25, 2, :],
            start=False, stop=True,
        )
        nc.vector.tensor_scalar_max(out_sb[0:N, 0:N], psfc[:, 0:N], 0.0)

    # Output DMA after the tile context. Fire-and-forget: the ~2us
    # transfer finishes in the shadow of the compiler's fixed ~6.5us
    # semaphore-reset epilogue that runs after the last instruction, so
    # no engine ever waits on its completion. Stays on the sync ring:
    # the scalar HWDGE ring would be cold (first issue ~2x as long).
    nc.sync.dma_start(out=out[:, :], in_=out_sb[0:N, 0:N]).then_inc(out_sem, 16)

    nc.compile()
    return nc


def pack_inputs(
    adj_matrix, W1, cheb1_b, peep1, b1, W2, cheb2_b, peep2, b2, fc_w, fc_b
) -> dict:
    """Host-side weight packing: gather/concat + bias fold + bf16 cast."""
    f = np.float32

    def gate_blk(Wg, bias):  # [k+1, d] with the bias fold row
        return np.concatenate([Wg, bias[None, :]], axis=0).astype(BF)

    adjp = np.zeros((36, 36), dtype=f)
    adjp[0:35, 0:35] = adj_matrix
    adjp[35, 0:35] = 1.0

    w1pa_h = np.zeros((36, 316), dtype=BF)
    w1pa_h[:, 0:36] = adjp.astype(BF)
    w1pa_h[:, 36:176] = gate_blk(W1[0], cheb1_b[0] + b1[0])
    w1pa_h[:, 176:316] = gate_blk(W1[2], cheb1_b[2] + b1[2])
    w1pb_h = np.zeros((128, 150), dtype=BF)
    w1pb_h[0:36, 0:140] = gate_blk(W1[3], cheb1_b[3] + b1[3])
    aux_h = np.zeros((128, 5), dtype=f)
    aux_h[:, 0] = peep1[2][0:128]
    aux_h[0:12, 1] = peep1[2][128:140]
    aux_h[:, 2] = peep2[2][0:128]
    aux_h[:, 3] = peep2[2][128:256]
    aux_h[0:24, 4] = peep2[2][256:280]
    w1pb_h[:, 140:150] = np.ascontiguousarray(aux_h).view(BF)

    w2a_h = np.stack([W2[0][0:128], W2[2][0:128]], axis=1).astype(BF)
    w2b_h = W2[3][0:128][:, None, :].astype(BF)
    w2k1_h = np.stack(
        [gate_blk(W2[g][128:140], cheb2_b[g] + b2[g]) for g in GATES],
        axis=1,
    )  # [13, 3, 280]

    fcx = np.concatenate([fc_w, fc_b[None, :]], axis=0)  # [281, 35]
    fcw_h = np.zeros((128, 3, 36), dtype=BF)
    fcw_h[:, 0, 0:35] = fcx[0:128].astype(BF)
    fcw_h[:, 1, 0:35] = fcx[128:256].astype(BF)
    fcw_h[0:25, 2, 0:35] = fcx[256:281].astype(BF)

    return {
        "w1pa": np.ascontiguousarray(w1pa_h),
        "w1pb": np.ascontiguousarray(w1pb_h),
        "w2a": np.ascontiguousarray(w2a_h),
        "w2b": np.ascontiguousarray(w2b_h),
        "w2k1": np.ascontiguousarray(w2k1_h),
        "fcw": np.ascontiguousarray(fcw_h),
    }


_NC_CACHE: list = []


def kernel(
    adj_matrix,
    W1,
    cheb1_W,
    cheb1_b,
    peep1,
    b1,
    W2,
    cheb2_W,
    cheb2_b,
    peep2,
    b2,
    fc_w,
    fc_b,
) -> np.ndarray:
    from concourse.bass_utils import run_bass_kernel_spmd

    in_map = pack_inputs(
        adj_matrix, W1, cheb1_b, peep1, b1, W2, cheb2_b, peep2, b2, fc_w, fc_b
    )

    if not _NC_CACHE:
        _NC_CACHE.append(build_nc())
    nc = _NC_CACHE[0]

    in_maps = [dict(in_map) for _ in range(N_CORES)]
    try:
        res = run_bass_kernel_spmd(nc, in_maps, core_ids=list(range(N_CORES)))
    except Exception:
        # transient device wedges (NRT_EXEC_*) usually clear on re-run
        res = run_bass_kernel_spmd(nc, in_maps, core_ids=list(range(N_CORES)))
    return np.asarray(res.results[0]["out"], dtype=np.float32)



# revision 2
# speedup vs baseline: 1.0723x; 1.0723x over previous
"""Trainium2 Bass kernel for nn_GCLSTMModel_48868137894020 (v3).

Same algebraic reduction as v1/v2 (H0 = C0 = 0 kills the cheb convs, the
forget gate, and peep[0/1]); per layer (d = 140 then 280), X = input:
  I = sigmoid(X @ W[0] + cb[0] + b[0])
  T = tanh   (X @ W[2] + cb[2] + b[2])
  C = I * T
  O = sigmoid(X @ W[3] + cb[3] + b[3] + peep[2] * C)
  X' = relu(O * tanh(C)) == max(tanh(C), 0) * O
then out = relu(X'' @ fc_w + fc_b).

v3 = v2 layout (everything transposed: features on partitions, nodes on
the free dim; bf16 matmuls; biases folded via a ones row) with the DMA
plan rebuilt from the v2 trace:
  - v2 issued 6 weight DMAs on the single sync HWDGE ring; the L1 O-gate
    weights (DMA #2) only landed at ~11.8us and the whole L1 -> L2 chain
    was DMA-paced, not compute-paced.
  - v3 packs ALL L1 weights + adj + aux scalars into ONE [128, 466] bf16
    transfer on the sync ring (lands ~2.3us after issue), with w2k1
    right behind it, and ALL L2 + FC weights into ONE [128, 948] bf16
    transfer on the scalar HWDGE ring (qActDynamicHW) which issues in
    parallel.  Two rings issue concurrently, so every weight is on-chip
    by ~9.5us instead of ~14.7us.
  - The ACT-queue order is: wb DMA issue first, then the activation
    warm-ups (whose auto-inserted table load runs in the DMA shadow).

Sharding: problem is tiny (N=35); all 8 cores run the identical program
on replicated inputs (no collectives), output taken from core 0.
"""

import sys

for _p in ("/opt/trn_rl_repo", "/opt/pypackages"):
    if _p not in sys.path:
        sys.path.append(_p)

from contextlib import ExitStack

import numpy as np
import ml_dtypes

import concourse.bacc as bacc
import concourse.bass as bass
import concourse.mybir as mybir
import concourse.tile as tile

F32 = mybir.dt.float32
BF16 = mybir.dt.bfloat16
AF = mybir.ActivationFunctionType
OP = mybir.AluOpType
GATES = (0, 2, 3)  # I, T (cell), O - forget gate (1) is dead
N = 35
D1 = 140
D2 = 280
N_CORES = 8
BF = ml_dtypes.bfloat16

# L1 feature chunks (140 = 128 + 12), L2 chunks (280 = 128 + 256 + 24).
C1 = ((0, 128), (128, 140))
C2 = ((0, 128), (128, 256), (256, 280))


def build_nc() -> bass.Bass:
    nc = bacc.Bacc()

    # wpk: [128, 466] bf16 - ALL layer-1 weights in one transfer:
    #   cols 0:36    rows 0:36 = [adj; ones] (k=36 rhs for L1)
    #   cols 36:176  rows 0:36 = [W1[I]; bias]
    #   cols 176:316 rows 0:36 = [W1[T]; bias]
    #   cols 316:456 rows 0:36 = [W1[O]; bias]
    #   cols 456:466 rows 0:128 = peep1[2]/peep2[2] per-partition scalars
    #                as a bit-cast fp32 [128, 5] block
    wpk = nc.dram_tensor("wpk", [128, 466], BF16, kind="ExternalInput")
    # W2 k-chunk1 (input features 128:140 + bias row): [13, 3, 280]
    w2k1 = nc.dram_tensor("w2k1", [13, 3, D2], BF16, kind="ExternalInput")
    # wb: [128, 948] bf16 - ALL layer-2 + FC weights (k-chunk0):
    #   cols 0:280 = W2[I], 280:560 = W2[T], 560:840 = W2[O],
    #   cols 840:948 = [fc_w; fc_b] row-chunks as [128, 3, 36]
    wb = nc.dram_tensor("wb", [128, 948], BF16, kind="ExternalInput")
    out = nc.dram_tensor("out", [N, N], F32, kind="ExternalOutput")

    # persistent sbuf tensor: written by the relu inside the tile
    # context, DMA'd out AFTER the context (ordered by the context-exit
    # all-engine barrier) with a semaphore nothing waits on.
    out_sb = nc.alloc_sbuf_tensor("out_sbuf", [N, N], F32)
    out_sem = nc.alloc_semaphore("out_dma_sem")

    with ExitStack() as ctx:
        tc = ctx.enter_context(tile.TileContext(nc))
        sb = ctx.enter_context(tc.tile_pool(name="sb", bufs=1))
        ps1 = ctx.enter_context(tc.tile_pool(name="ps1", bufs=3, space="PSUM"))
        ps2 = ctx.enter_context(tc.tile_pool(name="ps2", bufs=4, space="PSUM"))

        # ---- input DMAs: two HWDGE rings issue in parallel ----
        # sync ring: the L1 pack, then the small w2k1; scalar ring: the
        # big L2+FC pack (first thing on the ACT queue, before the
        # warm-up activations trigger the table load).
        wpk_sb = sb.tile([128, 466], BF16, tag="wpk")
        nc.sync.dma_start(out=wpk_sb, in_=wpk[:, :])
        wb_sb = sb.tile([128, 948], BF16, tag="wb")
        nc.scalar.dma_start(out=wb_sb, in_=wb[:, :])
        w2k1_sb = sb.tile([13, 3, D2], BF16, tag="w2k1")
        nc.sync.dma_start(out=w2k1_sb, in_=w2k1[:, :, :])

        adjp = wpk_sb[0:36, 0:36]
        aux_v = wpk_sb[:, 456:466].bitcast(F32)  # [128, 5] peep scalars

        # warm-up activations with no DMA deps: force the act-table
        # load to run during the DMA wait.
        warm_src = sb.tile([1, 2], F32, tag="warm_src")
        nc.vector.memset(warm_src[:, :], 0.25)
        warm = sb.tile([1, 2], F32, tag="warm")
        nc.scalar.activation(warm[0:1, 0:1], warm_src[0:1, 0:1], AF.Sigmoid)
        nc.scalar.activation(warm[0:1, 1:2], warm_src[0:1, 1:2], AF.Tanh)

        # ones rows for the bias folds: memset the whole chunk region
        # (partition offset must be 0-aligned); the gate STT later
        # overwrites all but the last row.
        x1T = sb.tile([128, 72], BF16, tag="x1T")
        nc.vector.memset(x1T[0:13, 36:72], 1.0)
        x2T = sb.tile([128, 108], BF16, tag="x2T")
        nc.vector.memset(x2T[0:25, 72:108], 1.0)

        # ---- layer 1 (transposed): psum banks [128, 72] ----
        p1 = {
            g: ps1.tile([128, 72], F32, tag="ps1", name=f"p1_{g}")
            for g in range(3)
        }
        l1_lhs = (
            lambda a, b: wpk_sb[0:36, 36 + a : 36 + b],
            lambda a, b: wpk_sb[0:36, 176 + a : 176 + b],
            lambda a, b: wpk_sb[0:36, 316 + a : 316 + b],
        )
        for g in range(3):
            for ci, (a, b) in enumerate(C1):
                nc.tensor.matmul(
                    p1[g][0 : b - a, ci * 36 : ci * 36 + 36],
                    lhsT=l1_lhs[g](a, b),
                    rhs=adjp,
                    start=True,
                    stop=True,
                )
        gi1 = sb.tile([128, 72], BF16, tag="gi1")
        nc.scalar.activation(gi1, p1[0][:, :], AF.Sigmoid)
        gt1 = sb.tile([128, 72], BF16, tag="gt1")
        nc.scalar.activation(gt1, p1[1][:, :], AF.Tanh)
        c1 = sb.tile([128, 72], BF16, tag="c1")
        nc.vector.tensor_mul(c1, gi1, gt1)
        tc1 = sb.tile([128, 72], BF16, tag="tc1")
        nc.scalar.activation(tc1, c1, AF.Tanh)
        po1 = sb.tile([128, 72], BF16, tag="po1")
        for ci, (a, b) in enumerate(C1):
            cs = b - a
            nc.vector.scalar_tensor_tensor(
                po1[0:cs, ci * 36 : ci * 36 + 36],
                in0=c1[0:cs, ci * 36 : ci * 36 + 36],
                scalar=aux_v[0:cs, ci : ci + 1],
                in1=p1[2][0:cs, ci * 36 : ci * 36 + 36],
                op0=OP.mult, op1=OP.add,
            )
        # O sigmoid split per chunk: chunk0's fires as soon as its STT
        # lands, so x1T chunk0 (the big L2 matmul rhs) is ready earlier
        go1 = sb.tile([128, 72], BF16, tag="go1")
        nc.scalar.activation(go1[0:128, 0:36], po1[0:128, 0:36], AF.Sigmoid)
        nc.scalar.activation(go1[0:12, 36:72], po1[0:12, 36:72], AF.Sigmoid)
        # x1T = max(tanh(C), 0) * O
        nc.vector.scalar_tensor_tensor(
            x1T[0:128, 0:36], in0=tc1[0:128, 0:36], scalar=0.0,
            in1=go1[0:128, 0:36], op0=OP.max, op1=OP.mult,
        )
        nc.vector.scalar_tensor_tensor(
            x1T[0:12, 36:72], in0=tc1[0:12, 36:72], scalar=0.0,
            in1=go1[0:12, 36:72], op0=OP.max, op1=OP.mult,
        )

        # ---- layer 2 (transposed): psum banks [128, 108] ----
        p2 = {
            g: ps2.tile([128, 108], F32, tag="ps2", name=f"p2_{g}")
            for g in range(3)
        }
        w2k0_views = (wb_sb[:, 0:280], wb_sb[:, 280:560], wb_sb[:, 560:840])
        # ONE open accumulation group per PSUM bank: close each region
        # (k0 then k1) before opening the next region in that bank.
        for g in (0, 1, 2):
            for ci, (a, b) in enumerate(C2):
                nc.tensor.matmul(
                    p2[g][0 : b - a, ci * 36 : ci * 36 + 36],
                    lhsT=w2k0_views[g][:, a:b],
                    rhs=x1T[0:128, 0:36],
                    start=True,
                    stop=False,
                )
                nc.tensor.matmul(
                    p2[g][0 : b - a, ci * 36 : ci * 36 + 36],
                    lhsT=w2k1_sb[0:13, g, a:b],
                    rhs=x1T[0:13, 36:72],
                    start=False,
                    stop=True,
                )
        # every gate stage split into an A piece (chunks 0+1, ready one
        # third of the MM stream earlier) and a small c2 piece, so the
        # gate chain overlaps the tail of the matmul stream
        A = (slice(0, 128), slice(0, 72))
        B = (slice(0, 24), slice(72, 108))
        gi2 = sb.tile([128, 108], BF16, tag="gi2")
        nc.scalar.activation(gi2[A], p2[0][A], AF.Sigmoid)
        nc.scalar.activation(gi2[B], p2[0][B], AF.Sigmoid)
        gt2 = sb.tile([128, 108], BF16, tag="gt2")
        nc.scalar.activation(gt2[A], p2[1][A], AF.Tanh)
        nc.scalar.activation(gt2[B], p2[1][B], AF.Tanh)
        c2 = sb.tile([128, 108], BF16, tag="c2")
        nc.vector.tensor_mul(c2[A], gi2[A], gt2[A])
        nc.vector.tensor_mul(c2[B], gi2[B], gt2[B])
        tc2 = sb.tile([128, 108], BF16, tag="tc2")
        nc.scalar.activation(tc2[A], c2[A], AF.Tanh)
        nc.scalar.activation(tc2[B], c2[B], AF.Tanh)
        po2 = sb.tile([128, 108], BF16, tag="po2")
        for ci, (a, b) in enumerate(C2):
            cs = b - a
            nc.vector.scalar_tensor_tensor(
                po2[0:cs, ci * 36 : ci * 36 + 36],
                in0=c2[0:cs, ci * 36 : ci * 36 + 36],
                scalar=aux_v[0:cs, 2 + ci : 3 + ci],
                in1=p2[2][0:cs, ci * 36 : ci * 36 + 36],
                op0=OP.mult, op1=OP.add,
            )
        go2 = sb.tile([128, 108], BF16, tag="go2")
        nc.scalar.activation(go2[A], po2[A], AF.Sigmoid)
        nc.scalar.activation(go2[B], po2[B], AF.Sigmoid)
        nc.vector.scalar_tensor_tensor(
            x2T[A], in0=tc2[A], scalar=0.0, in1=go2[A],
            op0=OP.max, op1=OP.mult,
        )
        nc.vector.scalar_tensor_tensor(
            x2T[B], in0=tc2[B], scalar=0.0, in1=go2[B],
            op0=OP.max, op1=OP.mult,
        )
        psfc = ps2.tile([N, 36], F32, tag="ps2", name="psfc")
        nc.tensor.matmul(
            psfc, lhsT=x2T[0:128, 0:35], rhs=wb_sb[:, 840:876],
            start=True, stop=False,
        )
        nc.tensor.matmul(
            psfc, lhsT=x2T[0:128, 36:71], rhs=wb_sb[:, 876:912],
            start=False, stop=False,
        )
        nc.tensor.matmul(
            psfc, lhsT=x2T[0:25, 72:107], rhs=wb_sb[0:25, 912:948],
            start=False, stop=True,
        )
        nc.vector.tensor_scalar_max(out_sb[0:N, 0:N], psfc[:, 0:N], 0.0)

    # Output DMA after the tile context. Fire-and-forget: the ~2us
    # transfer finishes in the shadow of the compiler's fixed ~6.5us
    # semaphore-reset epilogue that runs after the last instruction, so
    # no engine ever waits on its completion.
    nc.sync.dma_start(out=out[:, :], in_=out_sb[0:N, 0:N]).then_inc(out_sem, 16)

    nc.compile()
    return nc


def pack_inputs(
    adj_matrix, W1, cheb1_b, peep1, b1, W2, cheb2_b, peep2, b2, fc_w, fc_b
) -> dict:
    """Host-side weight packing: gather/concat + bias fold + bf16 cast."""
    f = np.float32

    def gate_blk(Wg, bias):  # [k+1, d] with the bias fold row
        return np.concatenate([Wg, bias[None, :]], axis=0).astype(BF)

    adjp = np.zeros((36, 36), dtype=f)
    adjp[0:35, 0:35] = adj_matrix
    adjp[35, 0:35] = 1.0

    wpk_h = np.zeros((128, 466), dtype=BF)
    wpk_h[0:36, 0:36] = adjp.astype(BF)
    wpk_h[0:36, 36:176] = gate_blk(W1[0], cheb1_b[0] + b1[0])
    wpk_h[0:36, 176:316] = gate_blk(W1[2], cheb1_b[2] + b1[2])
    wpk_h[0:36, 316:456] = gate_blk(W1[3], cheb1_b[3] + b1[3])
    aux_h = np.zeros((128, 5), dtype=f)
    aux_h[:, 0] = peep1[2][0:128]
    aux_h[0:12, 1] = peep1[2][128:140]
    aux_h[:, 2] = peep2[2][0:128]
    aux_h[:, 3] = peep2[2][128:256]
    aux_h[0:24, 4] = peep2[2][256:280]
    wpk_h[:, 456:466] = np.ascontiguousarray(aux_h).view(BF)

    w2k1_h = np.stack(
        [gate_blk(W2[g][128:140], cheb2_b[g] + b2[g]) for g in GATES],
        axis=1,
    )  # [13, 3, 280]

    wb_h = np.zeros((128, 948), dtype=BF)
    wb_h[:, 0:280] = W2[0][0:128].astype(BF)
    wb_h[:, 280:560] = W2[2][0:128].astype(BF)
    wb_h[:, 560:840] = W2[3][0:128].astype(BF)
    fcx = np.concatenate([fc_w, fc_b[None, :]], axis=0)  # [281, 35]
    wb_h[:, 840:875] = fcx[0:128].astype(BF)
    wb_h[:, 876:911] = fcx[128:256].astype(BF)
    wb_h[0:25, 912:947] = fcx[256:281].astype(BF)

    return {
        "wpk": np.ascontiguousarray(wpk_h),
        "w2k1": np.ascontiguousarray(w2k1_h),
        "wb": np.ascontiguousarray(wb_h),
    }


_NC_CACHE: list = []


def kernel(
    adj_matrix,
    W1,
    cheb1_W,
    cheb1_b,
    peep1,
    b1,
    W2,
    cheb2_W,
    cheb2_b,
    peep2,
    b2,
    fc_w,
    fc_b,
) -> np.ndarray:
    from concourse.bass_utils import run_bass_kernel_spmd

    in_map = pack_inputs(
        adj_matrix, W1, cheb1_b, peep1, b1, W2, cheb2_b, peep2, b2, fc_w, fc_b
    )

    if not _NC_CACHE:
        _NC_CACHE.append(build_nc())
    nc = _NC_CACHE[0]

    in_maps = [dict(in_map) for _ in range(N_CORES)]
    try:
        res = run_bass_kernel_spmd(nc, in_maps, core_ids=list(range(N_CORES)))
    except Exception:
        # transient device wedges (NRT_EXEC_*) usually clear on re-run
        res = run_bass_kernel_spmd(nc, in_maps, core_ids=list(range(N_CORES)))
    return np.asarray(res.results[0]["out"], dtype=np.float32)
